# revision 19
# baseline (speedup 1.0000x reference)
"""Multi-head attention Trainium2 kernel (8 NeuronCores, SPMD).

Problem: B=4, S=2048, D=1024, H=16, HD=64 dense MHA with 0/1 mask applied
to scores BEFORE softmax (masked score -> 0, so exp -> 1).

Sharding: core c handles batch b = c//2 and query-row half qh = c%2
(1024 queries). K/V are computed per-batch on both cores of a batch pair
(duplicated) so NO collectives are needed; each core holds every head for
its query rows, so the Wo projection is fully local.

Per-core algorithm (all matmuls bf16, f32 PSUM accumulation):
  KT[e,k] / QT[e,q] projections in transposed layout (head pairs packed
  to 128 partitions). Scores are computed transposed ([k, q]) for a head
  PAIR into one combined PSUM tile [128, h0|h1 x 512]: the two K=64
  matmuls hit disjoint PE row groups (partitions 0-63 / 64-127) and
  disjoint PSUM banks, so they execute concurrently, and a single
  exp + single copy_predicated (masked -> 1.0, exact semantics; the
  inverted-mask tile is column-doubled on the host to cover both heads)
  serve the whole pair. V is kept in natural [k,he] layout with a ones
  column per head (V') so the softmax denominator falls out of the
  attn@V matmul (zT row 64). attn@V runs in zT orientation (lhsT=V'
  chunk, rhs=masked-exp slice, N=512) - no transposes anywhere. All
  biases are applied by K=1 ones-vector matmuls accumulating into the
  projection PSUM. Normalization: denominator row broadcast across 64
  partitions via a K=1 matmul, fast approximate reciprocal (18-bit),
  multiply on GpSimd. Output projection consumes zT directly as lhsT.
"""

import sys

sys.path.insert(0, "/opt/trn_rl_repo")

import numpy as np
import ml_dtypes

import concourse.bass as bass
import concourse.mybir as mybir
import concourse.tile as tile
from concourse import bacc
from concourse.bass_utils import run_bass_kernel_spmd

BF16 = ml_dtypes.bfloat16

B, S, D, H, HD = 4, 2048, 1024, 16, 64
QW = 1024          # queries per core
SK = 2048          # keys per core
NPAIR = 8          # head pairs (2 heads of 64 -> 128 partitions)
DC = 8             # contraction chunks of 128 over D
KC = 16            # key chunks of 128
VW = 65            # V width incl. ones column
QB = 512           # query block (per head) in the paired scores tile
N_CORES = 8

_CACHED_NC = None


def _build_nc():
    dt = mybir.dt
    f32, b16 = dt.float32, dt.bfloat16
    Copy = mybir.ActivationFunctionType.Copy
    Exp = mybir.ActivationFunctionType.Exp
    Alu = mybir.AluOpType

    nc = bacc.Bacc("TRN2", target_bir_lowering=False, debug=False)

    d_xqt = nc.dram_tensor("xqt", [D, QW], b16, kind="ExternalInput").ap()
    d_xkt = nc.dram_tensor("xkt", [D, SK], b16, kind="ExternalInput").ap()
    d_xvt = nc.dram_tensor("xvt", [D, SK], b16, kind="ExternalInput").ap()
    # inverted mask, transposed, each 512-wide q block doubled (h0|h1)
    d_im2 = nc.dram_tensor("im2", [SK, 2 * QW], dt.uint8, kind="ExternalInput").ap()
    d_wq = nc.dram_tensor("wq", [D, D], b16, kind="ExternalInput").ap()
    d_wk = nc.dram_tensor("wk", [D, D], b16, kind="ExternalInput").ap()
    d_wv = nc.dram_tensor("wv", [D, D], b16, kind="ExternalInput").ap()
    d_wo = nc.dram_tensor("wo", [D, D], b16, kind="ExternalInput").ap()
    d_bq = nc.dram_tensor("bq", [1, NPAIR * 128], b16, kind="ExternalInput").ap()
    d_bk = nc.dram_tensor("bk", [1, NPAIR * 128], b16, kind="ExternalInput").ap()
    d_bv = nc.dram_tensor("bv", [1, D], b16, kind="ExternalInput").ap()
    d_bob = nc.dram_tensor("bob", [128, D], f32, kind="ExternalInput").ap()
    d_out = nc.dram_tensor("out", [QW, D], f32, kind="ExternalOutput").ap()

    with tile.TileContext(nc) as tc:
        # Keep single-tile free closures alive and idempotent: pools are a
        # LIFO bump allocator per space/side and a GC-run release corrupts
        # the stack order (or lands after scheduling).
        _keep = []

        def single(shape, dtype, name):
            t, free = tc.tile(shape, dtype, name=name)
            done = [False]

            def free_once():
                if not done[0]:
                    done[0] = True
                    free()

            _keep.append(free_once)
            return t, free_once

        # ---------------- persistent SBUF tiles ----------------
        kt, _ = single([128, NPAIR * SK], b16, "kt")       # [pair-e, k] per pair
        qt_, _ = single([128, NPAIR * QW], b16, "qt")      # [pair-e, q] per pair
        vp, _ = single([128, KC * H * VW], b16, "vp")      # [k-chunk, h*65] per chunk
        ones1, _ = single([1, 128], b16, "ones1")          # K=1 lhsT for bias matmuls
        onew, _ = single([128, 2 * QB], b16, "onew")       # ones data for masking
        bqp, _ = single([1, NPAIR * 128], b16, "bqp")
        bkp, _ = single([1, NPAIR * 128], b16, "bkp")
        bvr, _ = single([1, D], b16, "bvr")
        bob_sb, _ = single([128, D], f32, "bob_sb")

        nc.vector.memset(ones1[:], 1.0)
        nc.vector.memset(onew[:], 1.0)
        # ones columns of V' (position 64 of each 65-wide head block)
        nc.vector.memset(vp[:, 64::65], 1.0)

        nc.sync.dma_start(bqp[:], d_bq[:])
        nc.sync.dma_start(bkp[:], d_bk[:])
        nc.sync.dma_start(bvr[:], d_bv[:])
        nc.sync.dma_start(bob_sb[:], d_bob[:])

        # Phase-chained big buffers; frees must be LIFO per space/side.
        xk_sb, xk_free = single([128, DC * SK], b16, "xk_sb")
        wk_sb, wk_free = single([128, DC * D], b16, "wk_sb")
        xv_sb, xv_free = single([128, DC * SK], b16, "xv_sb")
        wv_sb, wv_free = single([128, DC * D], b16, "wv_sb")

        for dc in range(DC):
            nc.sync.dma_start(xv_sb[:, dc * SK:(dc + 1) * SK], d_xvt[dc * 128:(dc + 1) * 128, :])
            nc.sync.dma_start(wv_sb[:, dc * D:(dc + 1) * D], d_wv[dc * 128:(dc + 1) * 128, :])
            nc.sync.dma_start(xk_sb[:, dc * SK:(dc + 1) * SK], d_xkt[dc * 128:(dc + 1) * 128, :])
            nc.sync.dma_start(wk_sb[:, dc * D:(dc + 1) * D], d_wk[dc * 128:(dc + 1) * 128, :])

        # ---------------- projections ----------------
        with tc.tile_pool(name="proj_ps", space="PSUM", bufs=4) as proj_pool:
            # V projection: V'[s, h*65:h*65+64] = xv.T chunks @ Wv + bv
            for sc in range(KC):
                for nh in range(2):
                    ps = proj_pool.tile([128, 512], f32, tag="ps")
                    nc.tensor.matmul(  # bias: ones[s] x bv[he]
                        ps[:], lhsT=ones1[:, 0:128],
                        rhs=bvr[:, nh * 512:(nh + 1) * 512],
                        start=True, stop=False,
                    )
                    for dc in range(DC):
                        nc.tensor.matmul(
                            ps[:],
                            lhsT=xv_sb[:, dc * SK + sc * 128: dc * SK + (sc + 1) * 128],
                            rhs=wv_sb[:, dc * D + nh * 512: dc * D + (nh + 1) * 512],
                            start=False, stop=(dc == DC - 1),
                        )
                    # scatter 8 heads x 64 into the 65-strided V' block
                    o3 = vp[:, sc * H * VW + nh * 8 * VW: sc * H * VW + (nh * 8 + 8) * VW]
                    o3 = o3.rearrange("p (h c) -> p h c", h=8)[:, :, 0:64]
                    i3 = ps[:].rearrange("p (h c) -> p h c", h=8)
                    nc.scalar.activation(o3, i3, Copy)
            wv_free()
            xv_free()

            # K projection -> KT [pair-e 128, k]
            for p in range(NPAIR):
                for ns in range(SK // 512):
                    ps = proj_pool.tile([128, 512], f32, tag="ps")
                    nc.tensor.matmul(  # bias: bk[e] x ones[k]
                        ps[:], lhsT=bkp[0:1, p * 128:(p + 1) * 128], rhs=onew[0:1, 0:512],
                        start=True, stop=False,
                    )
                    for dc in range(DC):
                        nc.tensor.matmul(
                            ps[:],
                            lhsT=wk_sb[:, dc * D + p * 128: dc * D + (p + 1) * 128],
                            rhs=xk_sb[:, dc * SK + ns * 512: dc * SK + (ns + 1) * 512],
                            start=False, stop=(dc == DC - 1),
                        )
                    nc.scalar.activation(
                        kt[:, p * SK + ns * 512: p * SK + (ns + 1) * 512], ps[:], Copy,
                    )
            wk_free()
            xk_free()

            xq_sb, xq_free = single([128, DC * QW], b16, "xq_sb")
            wq_sb, wq_free = single([128, DC * D], b16, "wq_sb")
            for dc in range(DC):
                nc.sync.dma_start(xq_sb[:, dc * QW:(dc + 1) * QW], d_xqt[dc * 128:(dc + 1) * 128, :])
                nc.sync.dma_start(wq_sb[:, dc * D:(dc + 1) * D], d_wq[dc * 128:(dc + 1) * 128, :])

            # Q projection -> QT [pair-e 128, q]
            for p in range(NPAIR):
                for ns in range(QW // 512):
                    ps = proj_pool.tile([128, 512], f32, tag="ps")
                    nc.tensor.matmul(
                        ps[:], lhsT=bqp[0:1, p * 128:(p + 1) * 128], rhs=onew[0:1, 0:512],
                        start=True, stop=False,
                    )
                    for dc in range(DC):
                        nc.tensor.matmul(
                            ps[:],
                            lhsT=wq_sb[:, dc * D + p * 128: dc * D + (p + 1) * 128],
                            rhs=xq_sb[:, dc * QW + ns * 512: dc * QW + (ns + 1) * 512],
                            start=False, stop=(dc == DC - 1),
                        )
                    nc.scalar.activation(
                        qt_[:, p * QW + ns * 512: p * QW + (ns + 1) * 512], ps[:], Copy,
                    )
            wq_free()
            xq_free()

        # column-doubled inverted mask into the space freed by x buffers
        im_sb, _ = single([128, KC * 2 * QW], dt.uint8, "im_sb")
        for kc in range(KC):
            nc.sync.dma_start(
                im_sb[:, kc * 2 * QW:(kc + 1) * 2 * QW],
                d_im2[kc * 128:(kc + 1) * 128, :],
            )
        wo_sb, _ = single([128, NPAIR * D], b16, "wo_sb")
        for p in range(NPAIR):
            nc.sync.dma_start(wo_sb[:, p * D:(p + 1) * D], d_wo[p * 128:(p + 1) * 128, :])
        zt, _ = single([128, NPAIR * QW], b16, "zt")       # [pair-he, q]

        # ---------------- attention ----------------
        with (
            tc.tile_pool(name="sc_ps", space="PSUM", bufs=2) as sc_pool,
            tc.tile_pool(name="zt_ps", space="PSUM", bufs=2) as zt_pool,
            tc.tile_pool(name="att_sb", bufs=6) as att_pool,
            tc.tile_pool(name="ep_sb", bufs=3) as ep_pool,
        ):
            for p in range(NPAIR):
                for qb in range(QW // QB):
                    q0 = p * QW + qb * QB
                    zt_ps = [
                        zt_pool.tile([VW, QB], f32, name=f"ztp{hi}", tag=f"ztp{hi}")
                        for hi in range(2)
                    ]
                    for kc in range(KC):
                        # paired scores: both heads' K=64 matmuls in one
                        # combined tile - disjoint row groups AND banks,
                        # gated by the same slot event -> concurrent
                        sc2 = sc_pool.tile([128, 2 * QB], f32, tag="sc")
                        for hi in range(2):
                            r0, r1 = hi * 64, (hi + 1) * 64
                            nc.tensor.matmul(
                                sc2[:, hi * QB:(hi + 1) * QB],
                                lhsT=kt[r0:r1, p * SK + kc * 128: p * SK + (kc + 1) * 128],
                                rhs=qt_[r0:r1, q0: q0 + QB],
                                start=True, stop=True,
                            )
                        e2 = att_pool.tile([128, 2 * QB], b16, tag="e")
                        nc.scalar.activation(e2[:], sc2[:], Exp)
                        # masked positions (inverted mask nonzero) -> 1.0
                        nc.vector.copy_predicated(
                            e2[:],
                            im_sb[:, kc * 2 * QW + qb * 2 * QB: kc * 2 * QW + (qb + 1) * 2 * QB],
                            onew[:],
                        )
                        for hi in range(2):
                            h = 2 * p + hi
                            nc.tensor.matmul(
                                zt_ps[hi][:],
                                lhsT=vp[:, kc * H * VW + h * VW: kc * H * VW + (h + 1) * VW],
                                rhs=e2[:, hi * QB:(hi + 1) * QB],
                                start=(kc == 0), stop=(kc == KC - 1),
                            )
                    # epilogue: move zT out of PSUM fast, normalize from SBUF
                    for hi in range(2):
                        zu = ep_pool.tile([VW, QB], f32, name=f"zu{hi}", tag=f"zu{hi}")
                        nc.scalar.activation(zu[:], zt_ps[hi][:], Copy)
                        den_b = ep_pool.tile([1, QB], b16, tag="denb")
                        nc.scalar.activation(den_b[:], zu[64:65, :], Copy)
                        db_ps = sc_pool.tile([128, 2 * QB], f32, tag="sc")
                        nc.tensor.matmul(
                            db_ps[0:64, 0:QB],
                            lhsT=ones1[:, 0:64], rhs=den_b[:],
                            start=True, stop=True,
                        )
                        rb_sb = ep_pool.tile([64, QB], f32, tag="rbsb")
                        nc.vector.reciprocal_approx_fast(rb_sb[:], db_ps[0:64, 0:QB])
                        nc.gpsimd.tensor_tensor(
                            zt[hi * 64:(hi + 1) * 64, q0: q0 + QB],
                            zu[0:64, :], rb_sb[:], op=Alu.mult,
                        )

        # ---------------- output projection ----------------
        with (
            tc.tile_pool(name="wo_ps", space="PSUM", bufs=2) as wo_pool,
            tc.tile_pool(name="out_sb", bufs=2) as out_pool,
        ):
            for jq in range(8):
                o_sb = out_pool.tile([128, D], f32, tag="o")
                for n in range(2):
                    ps = wo_pool.tile([128, 512], f32, tag="wo")
                    for p in range(NPAIR):
                        nc.tensor.matmul(
                            ps[:],
                            lhsT=zt[:, p * QW + jq * 128: p * QW + (jq + 1) * 128],
                            rhs=wo_sb[:, p * D + n * 512: p * D + (n + 1) * 512],
                            start=(p == 0), stop=(p == NPAIR - 1),
                        )
                    nc.vector.scalar_tensor_tensor(
                        o_sb[:, n * 512:(n + 1) * 512], ps[:], 0.0,
                        bob_sb[:, n * 512:(n + 1) * 512],
                        op0=Alu.bypass, op1=Alu.add,
                    )
                nc.sync.dma_start(d_out[jq * 128:(jq + 1) * 128, :], o_sb[:])

        # Release remaining singles in LIFO order BEFORE TileContext exit,
        # else GC-driven releases append boundary pseudo-instructions to
        # the already-committed program (walrus aborts on them).
        for f in reversed(_keep):
            f()

    nc.compile()
    return nc


def get_nc():
    global _CACHED_NC
    if _CACHED_NC is None:
        _CACHED_NC = _build_nc()
    return _CACHED_NC


def _prep_in_maps(x_v, x_k, x_q, mask, Wq, bq, Wk, bk, Wv, bv, Wo, bo):
    """Host-side shard + layout prep. Cheap numpy transposes/casts only."""
    wq_f = (np.transpose(Wq, (1, 0, 2)).reshape(D, D) / 8.0).astype(BF16)
    wk_f = np.transpose(Wk, (1, 0, 2)).reshape(D, D).astype(BF16)
    wv_f = np.transpose(Wv, (1, 0, 2)).reshape(D, D).astype(BF16)
    wo_f = Wo.astype(BF16)
    bq_f = (bq.reshape(1, NPAIR * 128) / 8.0).astype(BF16)
    bk_f = bk.reshape(1, NPAIR * 128).astype(BF16)
    bv_f = bv.reshape(1, D).astype(BF16)
    bob = np.ascontiguousarray(np.broadcast_to(bo.reshape(1, D), (128, D)), dtype=np.float32)

    in_maps = []
    for c in range(N_CORES):
        b, qh = c // 2, c % 2
        q0 = qh * QW
        im = (1 - mask[b, q0:q0 + QW]).T.astype(np.uint8)     # [SK, QW]
        im2 = np.empty((SK, 2 * QW), np.uint8)                # each 512 block doubled
        for qb in range(QW // QB):
            blk = im[:, qb * QB:(qb + 1) * QB]
            im2[:, qb * 2 * QB: qb * 2 * QB + QB] = blk
            im2[:, qb * 2 * QB + QB: (qb + 1) * 2 * QB] = blk
        in_maps.append({
            "xqt": np.ascontiguousarray(x_q[b, q0:q0 + QW].T).astype(BF16),
            "xkt": np.ascontiguousarray(x_k[b].T).astype(BF16),
            "xvt": np.ascontiguousarray(x_v[b].T).astype(BF16),
            "im2": im2,
            "wq": wq_f, "wk": wk_f, "wv": wv_f, "wo": wo_f,
            "bq": bq_f, "bk": bk_f, "bv": bv_f, "bob": bob,
        })
    return in_maps


def _install_axon_ntff_hook():
    """The container's antenv stub lacks axon_hooks, so trace=True can't
    find the NTFF profile hook. Recreate the registry module and install
    the ctypes-based hook from trn_agent_boot against libaxon_pjrt.so."""
    import types

    if "antenv.axon_hooks" in sys.modules:
        return
    import antenv

    mod = types.ModuleType("antenv.axon_hooks")
    _hook = [None]
    mod.set_axon_ntff_profile_hook = lambda h: _hook.__setitem__(0, h)
    mod.get_axon_ntff_profile_hook = lambda: _hook[0]
    sys.modules["antenv.axon_hooks"] = mod
    antenv.axon_hooks = mod
    try:
        sys.path.insert(0, "/root/.axon_site")
        from trn_agent_boot.trn_boot import _ntff_profile_via_ctypes

        mod.set_axon_ntff_profile_hook(
            _ntff_profile_via_ctypes("/opt/axon/libaxon_pjrt.so")
        )
    except Exception as e:  # degrade to no-trace
        print(f"ntff hook install failed: {e}", file=sys.stderr)


def run(trace=False, **inputs):
    if trace:
        _install_axon_ntff_hook()
    nc = get_nc()
    in_maps = _prep_in_maps(**inputs)
    res = run_bass_kernel_spmd(nc, in_maps, core_ids=list(range(N_CORES)), trace=trace)
    out = np.zeros((B, S, D), np.float32)
    for c in range(N_CORES):
        b, qh = c // 2, c % 2
        out[b, qh * QW:(qh + 1) * QW, :] = res.results[c]["out"]
    return out, res


def kernel(**inputs):
    out, _ = run(trace=False, **inputs)
    return out


# revision 20
# speedup vs baseline: 1.0061x; 1.0061x over previous
"""Multi-head attention Trainium2 kernel (8 NeuronCores, SPMD).

Problem: B=4, S=2048, D=1024, H=16, HD=64 dense MHA with 0/1 mask applied
to scores BEFORE softmax (masked score -> 0, so exp -> 1).

Sharding: core c handles batch b = c//2 and query-row half qh = c%2
(1024 queries). K/V are computed per-batch on both cores of a batch pair
(duplicated) so NO collectives are needed; each core holds every head for
its query rows, so the Wo projection is fully local.

Per-core algorithm (all matmuls bf16, f32 PSUM accumulation):
  KT[e,k] / QT[e,q] projections in transposed layout (head pairs packed
  to 128 partitions). Scores are computed transposed ([k, q]) for a head
  PAIR into one combined PSUM tile [128, h0|h1 x 512]: the two K=64
  matmuls hit disjoint PE row groups (partitions 0-63 / 64-127) and
  disjoint PSUM banks, so they execute concurrently, and a single
  exp + single copy_predicated (masked -> 1.0, exact semantics; the
  inverted-mask tile is column-doubled on the host to cover both heads)
  serve the whole pair. V is kept in natural [k,he] layout with a ones
  column per head (V') so the softmax denominator falls out of the
  attn@V matmul (zT row 64). attn@V runs in zT orientation (lhsT=V'
  chunk, rhs=masked-exp slice, N=512) - no transposes anywhere. All
  biases are applied by K=1 ones-vector matmuls accumulating into the
  projection PSUM. Normalization: denominator row broadcast across 64
  partitions via a K=1 matmul, fast approximate reciprocal (18-bit),
  multiply on GpSimd. Output projection consumes zT directly as lhsT.
"""

import sys

sys.path.insert(0, "/opt/trn_rl_repo")

import numpy as np
import ml_dtypes

import concourse.bass as bass
import concourse.mybir as mybir
import concourse.tile as tile
from concourse import bacc
from concourse.bass_utils import run_bass_kernel_spmd

BF16 = ml_dtypes.bfloat16

B, S, D, H, HD = 4, 2048, 1024, 16, 64
QW = 1024          # queries per core
SK = 2048          # keys per core
NPAIR = 8          # head pairs (2 heads of 64 -> 128 partitions)
DC = 8             # contraction chunks of 128 over D
KC = 16            # key chunks of 128
VW = 65            # V width incl. ones column
QB = 512           # query block (per head) in the paired scores tile
N_CORES = 8

_CACHED_NC = None


def _build_nc():
    dt = mybir.dt
    f32, b16 = dt.float32, dt.bfloat16
    Copy = mybir.ActivationFunctionType.Copy
    Exp = mybir.ActivationFunctionType.Exp
    Alu = mybir.AluOpType

    nc = bacc.Bacc("TRN2", target_bir_lowering=False, debug=False)

    d_xqt = nc.dram_tensor("xqt", [D, QW], b16, kind="ExternalInput").ap()
    d_xkt = nc.dram_tensor("xkt", [D, SK], b16, kind="ExternalInput").ap()
    d_xvt = nc.dram_tensor("xvt", [D, SK], b16, kind="ExternalInput").ap()
    # inverted mask, transposed, each 512-wide q block doubled (h0|h1)
    d_im2 = nc.dram_tensor("im2", [SK, 2 * QW], dt.uint8, kind="ExternalInput").ap()
    d_wq = nc.dram_tensor("wq", [D, D], b16, kind="ExternalInput").ap()
    d_wk = nc.dram_tensor("wk", [D, D], b16, kind="ExternalInput").ap()
    d_wv = nc.dram_tensor("wv", [D, D], b16, kind="ExternalInput").ap()
    d_wo = nc.dram_tensor("wo", [D, D], b16, kind="ExternalInput").ap()
    d_bq = nc.dram_tensor("bq", [1, NPAIR * 128], b16, kind="ExternalInput").ap()
    d_bk = nc.dram_tensor("bk", [1, NPAIR * 128], b16, kind="ExternalInput").ap()
    d_bv = nc.dram_tensor("bv", [1, D], b16, kind="ExternalInput").ap()
    d_bob = nc.dram_tensor("bob", [128, D], f32, kind="ExternalInput").ap()
    d_out = nc.dram_tensor("out", [QW, D], f32, kind="ExternalOutput").ap()

    with tile.TileContext(nc) as tc:
        # Keep single-tile free closures alive and idempotent: pools are a
        # LIFO bump allocator per space/side and a GC-run release corrupts
        # the stack order (or lands after scheduling).
        _keep = []

        def single(shape, dtype, name):
            t, free = tc.tile(shape, dtype, name=name)
            done = [False]

            def free_once():
                if not done[0]:
                    done[0] = True
                    free()

            _keep.append(free_once)
            return t, free_once

        # ---------------- persistent SBUF tiles ----------------
        kt, _ = single([128, NPAIR * SK], b16, "kt")       # [pair-e, k] per pair
        qt_, _ = single([128, NPAIR * QW], b16, "qt")      # [pair-e, q] per pair
        vp, _ = single([128, KC * H * VW], b16, "vp")      # [k-chunk, h*65] per chunk
        ones1, _ = single([1, 128], b16, "ones1")          # K=1 lhsT for bias matmuls
        onew, _ = single([128, 2 * QB], b16, "onew")       # ones data for masking
        bqp, _ = single([1, NPAIR * 128], b16, "bqp")
        bkp, _ = single([1, NPAIR * 128], b16, "bkp")
        bvr, _ = single([1, D], b16, "bvr")
        bob_sb, _ = single([128, D], f32, "bob_sb")

        nc.vector.memset(ones1[:], 1.0)
        nc.vector.memset(onew[:], 1.0)
        # ones columns of V' (position 64 of each 65-wide head block)
        nc.vector.memset(vp[:, 64::65], 1.0)

        nc.sync.dma_start(bqp[:], d_bq[:])
        nc.sync.dma_start(bkp[:], d_bk[:])
        nc.sync.dma_start(bvr[:], d_bv[:])
        nc.sync.dma_start(bob_sb[:], d_bob[:])

        # Phase-chained big buffers; frees must be LIFO per space/side.
        xk_sb, xk_free = single([128, DC * SK], b16, "xk_sb")
        wk_sb, wk_free = single([128, DC * D], b16, "wk_sb")
        xv_sb, xv_free = single([128, DC * SK], b16, "xv_sb")
        wv_sb, wv_free = single([128, DC * D], b16, "wv_sb")

        for dc in range(DC):
            nc.sync.dma_start(xv_sb[:, dc * SK:(dc + 1) * SK], d_xvt[dc * 128:(dc + 1) * 128, :])
            nc.sync.dma_start(wv_sb[:, dc * D:(dc + 1) * D], d_wv[dc * 128:(dc + 1) * 128, :])
            nc.sync.dma_start(xk_sb[:, dc * SK:(dc + 1) * SK], d_xkt[dc * 128:(dc + 1) * 128, :])
            nc.sync.dma_start(wk_sb[:, dc * D:(dc + 1) * D], d_wk[dc * 128:(dc + 1) * 128, :])

        # ---------------- projections ----------------
        with tc.tile_pool(name="proj_ps", space="PSUM", bufs=4) as proj_pool:
            # V projection: V'[s, h*65:h*65+64] = xv.T chunks @ Wv + bv
            for sc in range(KC):
                for nh in range(2):
                    ps = proj_pool.tile([128, 512], f32, tag="ps")
                    nc.tensor.matmul(  # bias: ones[s] x bv[he]
                        ps[:], lhsT=ones1[:, 0:128],
                        rhs=bvr[:, nh * 512:(nh + 1) * 512],
                        start=True, stop=False,
                    )
                    for dc in range(DC):
                        nc.tensor.matmul(
                            ps[:],
                            lhsT=xv_sb[:, dc * SK + sc * 128: dc * SK + (sc + 1) * 128],
                            rhs=wv_sb[:, dc * D + nh * 512: dc * D + (nh + 1) * 512],
                            start=False, stop=(dc == DC - 1),
                        )
                    # scatter 8 heads x 64 into the 65-strided V' block
                    o3 = vp[:, sc * H * VW + nh * 8 * VW: sc * H * VW + (nh * 8 + 8) * VW]
                    o3 = o3.rearrange("p (h c) -> p h c", h=8)[:, :, 0:64]
                    i3 = ps[:].rearrange("p (h c) -> p h c", h=8)
                    nc.scalar.activation(o3, i3, Copy)
            wv_free()
            xv_free()

            # K projection -> KT [pair-e 128, k]
            for p in range(NPAIR):
                for ns in range(SK // 512):
                    ps = proj_pool.tile([128, 512], f32, tag="ps")
                    nc.tensor.matmul(  # bias: bk[e] x ones[k]
                        ps[:], lhsT=bkp[0:1, p * 128:(p + 1) * 128], rhs=onew[0:1, 0:512],
                        start=True, stop=False,
                    )
                    for dc in range(DC):
                        nc.tensor.matmul(
                            ps[:],
                            lhsT=wk_sb[:, dc * D + p * 128: dc * D + (p + 1) * 128],
                            rhs=xk_sb[:, dc * SK + ns * 512: dc * SK + (ns + 1) * 512],
                            start=False, stop=(dc == DC - 1),
                        )
                    nc.scalar.activation(
                        kt[:, p * SK + ns * 512: p * SK + (ns + 1) * 512], ps[:], Copy,
                    )
            wk_free()
            xk_free()

            xq_sb, xq_free = single([128, DC * QW], b16, "xq_sb")
            wq_sb, wq_free = single([128, DC * D], b16, "wq_sb")
            for dc in range(DC):
                nc.sync.dma_start(xq_sb[:, dc * QW:(dc + 1) * QW], d_xqt[dc * 128:(dc + 1) * 128, :])
                nc.sync.dma_start(wq_sb[:, dc * D:(dc + 1) * D], d_wq[dc * 128:(dc + 1) * 128, :])

            # Q projection -> QT [pair-e 128, q]
            for p in range(NPAIR):
                for ns in range(QW // 512):
                    ps = proj_pool.tile([128, 512], f32, tag="ps")
                    nc.tensor.matmul(
                        ps[:], lhsT=bqp[0:1, p * 128:(p + 1) * 128], rhs=onew[0:1, 0:512],
                        start=True, stop=False,
                    )
                    for dc in range(DC):
                        nc.tensor.matmul(
                            ps[:],
                            lhsT=wq_sb[:, dc * D + p * 128: dc * D + (p + 1) * 128],
                            rhs=xq_sb[:, dc * QW + ns * 512: dc * QW + (ns + 1) * 512],
                            start=False, stop=(dc == DC - 1),
                        )
                    nc.scalar.activation(
                        qt_[:, p * QW + ns * 512: p * QW + (ns + 1) * 512], ps[:], Copy,
                    )
            wq_free()
            xq_free()

        # column-doubled inverted mask into the space freed by x buffers
        im_sb, _ = single([128, KC * 2 * QW], dt.uint8, "im_sb")
        for kc in range(KC):
            nc.sync.dma_start(
                im_sb[:, kc * 2 * QW:(kc + 1) * 2 * QW],
                d_im2[kc * 128:(kc + 1) * 128, :],
            )
        wo_sb, _ = single([128, NPAIR * D], b16, "wo_sb")
        for p in range(NPAIR):
            nc.sync.dma_start(wo_sb[:, p * D:(p + 1) * D], d_wo[p * 128:(p + 1) * 128, :])
        zt, _ = single([128, NPAIR * QW], b16, "zt")       # [pair-he, q]

        # ---------------- attention ----------------
        with (
            tc.tile_pool(name="sc_ps", space="PSUM", bufs=3) as sc_pool,
            tc.tile_pool(name="zt_ps", space="PSUM", bufs=1) as zt_pool,
            tc.tile_pool(name="att_sb", bufs=6) as att_pool,
            tc.tile_pool(name="ep_sb", bufs=3) as ep_pool,
        ):
            for p in range(NPAIR):
                for qb in range(QW // QB):
                    q0 = p * QW + qb * QB
                    zt_ps = [
                        zt_pool.tile([VW, QB], f32, name=f"ztp{hi}", tag=f"ztp{hi}")
                        for hi in range(2)
                    ]
                    for kc in range(KC):
                        # paired scores: both heads' K=64 matmuls in one
                        # combined tile - disjoint row groups AND banks,
                        # gated by the same slot event -> concurrent
                        sc2 = sc_pool.tile([128, 2 * QB], f32, tag="sc")
                        for hi in range(2):
                            r0, r1 = hi * 64, (hi + 1) * 64
                            nc.tensor.matmul(
                                sc2[:, hi * QB:(hi + 1) * QB],
                                lhsT=kt[r0:r1, p * SK + kc * 128: p * SK + (kc + 1) * 128],
                                rhs=qt_[r0:r1, q0: q0 + QB],
                                start=True, stop=True,
                            )
                        e2 = att_pool.tile([128, 2 * QB], b16, tag="e")
                        nc.scalar.activation(e2[:], sc2[:], Exp)
                        # masked positions (inverted mask nonzero) -> 1.0
                        nc.vector.copy_predicated(
                            e2[:],
                            im_sb[:, kc * 2 * QW + qb * 2 * QB: kc * 2 * QW + (qb + 1) * 2 * QB],
                            onew[:],
                        )
                        for hi in range(2):
                            h = 2 * p + hi
                            nc.tensor.matmul(
                                zt_ps[hi][:],
                                lhsT=vp[:, kc * H * VW + h * VW: kc * H * VW + (h + 1) * VW],
                                rhs=e2[:, hi * QB:(hi + 1) * QB],
                                start=(kc == 0), stop=(kc == KC - 1),
                            )
                    # epilogue: move zT out of PSUM fast, normalize from SBUF
                    for hi in range(2):
                        zu = ep_pool.tile([VW, QB], f32, name=f"zu{hi}", tag=f"zu{hi}")
                        nc.scalar.activation(zu[:], zt_ps[hi][:], Copy)
                        den_b = ep_pool.tile([1, QB], b16, tag="denb")
                        nc.scalar.activation(den_b[:], zu[64:65, :], Copy)
                        db_ps = sc_pool.tile([128, 2 * QB], f32, tag="sc")
                        nc.tensor.matmul(
                            db_ps[0:64, 0:QB],
                            lhsT=ones1[:, 0:64], rhs=den_b[:],
                            start=True, stop=True,
                        )
                        rb_sb = ep_pool.tile([64, QB], f32, tag="rbsb")
                        nc.vector.reciprocal_approx_fast(rb_sb[:], db_ps[0:64, 0:QB])
                        nc.gpsimd.tensor_tensor(
                            zt[hi * 64:(hi + 1) * 64, q0: q0 + QB],
                            zu[0:64, :], rb_sb[:], op=Alu.mult,
                        )

        # ---------------- output projection ----------------
        with (
            tc.tile_pool(name="wo_ps", space="PSUM", bufs=2) as wo_pool,
            tc.tile_pool(name="out_sb", bufs=2) as out_pool,
        ):
            for jq in range(8):
                o_sb = out_pool.tile([128, D], f32, tag="o")
                for n in range(2):
                    ps = wo_pool.tile([128, 512], f32, tag="wo")
                    for p in range(NPAIR):
                        nc.tensor.matmul(
                            ps[:],
                            lhsT=zt[:, p * QW + jq * 128: p * QW + (jq + 1) * 128],
                            rhs=wo_sb[:, p * D + n * 512: p * D + (n + 1) * 512],
                            start=(p == 0), stop=(p == NPAIR - 1),
                        )
                    nc.vector.scalar_tensor_tensor(
                        o_sb[:, n * 512:(n + 1) * 512], ps[:], 0.0,
                        bob_sb[:, n * 512:(n + 1) * 512],
                        op0=Alu.bypass, op1=Alu.add,
                    )
                nc.sync.dma_start(d_out[jq * 128:(jq + 1) * 128, :], o_sb[:])

        # Release remaining singles in LIFO order BEFORE TileContext exit,
        # else GC-driven releases append boundary pseudo-instructions to
        # the already-committed program (walrus aborts on them).
        for f in reversed(_keep):
            f()

    nc.compile()
    return nc


def get_nc():
    global _CACHED_NC
    if _CACHED_NC is None:
        _CACHED_NC = _build_nc()
    return _CACHED_NC


def _prep_in_maps(x_v, x_k, x_q, mask, Wq, bq, Wk, bk, Wv, bv, Wo, bo):
    """Host-side shard + layout prep. Cheap numpy transposes/casts only."""
    wq_f = (np.transpose(Wq, (1, 0, 2)).reshape(D, D) / 8.0).astype(BF16)
    wk_f = np.transpose(Wk, (1, 0, 2)).reshape(D, D).astype(BF16)
    wv_f = np.transpose(Wv, (1, 0, 2)).reshape(D, D).astype(BF16)
    wo_f = Wo.astype(BF16)
    bq_f = (bq.reshape(1, NPAIR * 128) / 8.0).astype(BF16)
    bk_f = bk.reshape(1, NPAIR * 128).astype(BF16)
    bv_f = bv.reshape(1, D).astype(BF16)
    bob = np.ascontiguousarray(np.broadcast_to(bo.reshape(1, D), (128, D)), dtype=np.float32)

    in_maps = []
    for c in range(N_CORES):
        b, qh = c // 2, c % 2
        q0 = qh * QW
        im = (1 - mask[b, q0:q0 + QW]).T.astype(np.uint8)     # [SK, QW]
        im2 = np.empty((SK, 2 * QW), np.uint8)                # each 512 block doubled
        for qb in range(QW // QB):
            blk = im[:, qb * QB:(qb + 1) * QB]
            im2[:, qb * 2 * QB: qb * 2 * QB + QB] = blk
            im2[:, qb * 2 * QB + QB: (qb + 1) * 2 * QB] = blk
        in_maps.append({
            "xqt": np.ascontiguousarray(x_q[b, q0:q0 + QW].T).astype(BF16),
            "xkt": np.ascontiguousarray(x_k[b].T).astype(BF16),
            "xvt": np.ascontiguousarray(x_v[b].T).astype(BF16),
            "im2": im2,
            "wq": wq_f, "wk": wk_f, "wv": wv_f, "wo": wo_f,
            "bq": bq_f, "bk": bk_f, "bv": bv_f, "bob": bob,
        })
    return in_maps


def _install_axon_ntff_hook():
    """The container's antenv stub lacks axon_hooks, so trace=True can't
    find the NTFF profile hook. Recreate the registry module and install
    the ctypes-based hook from trn_agent_boot against libaxon_pjrt.so."""
    import types

    if "antenv.axon_hooks" in sys.modules:
        return
    import antenv

    mod = types.ModuleType("antenv.axon_hooks")
    _hook = [None]
    mod.set_axon_ntff_profile_hook = lambda h: _hook.__setitem__(0, h)
    mod.get_axon_ntff_profile_hook = lambda: _hook[0]
    sys.modules["antenv.axon_hooks"] = mod
    antenv.axon_hooks = mod
    try:
        sys.path.insert(0, "/root/.axon_site")
        from trn_agent_boot.trn_boot import _ntff_profile_via_ctypes

        mod.set_axon_ntff_profile_hook(
            _ntff_profile_via_ctypes("/opt/axon/libaxon_pjrt.so")
        )
    except Exception as e:  # degrade to no-trace
        print(f"ntff hook install failed: {e}", file=sys.stderr)


def run(trace=False, **inputs):
    if trace:
        _install_axon_ntff_hook()
    nc = get_nc()
    in_maps = _prep_in_maps(**inputs)
    res = run_bass_kernel_spmd(nc, in_maps, core_ids=list(range(N_CORES)), trace=trace)
    out = np.zeros((B, S, D), np.float32)
    for c in range(N_CORES):
        b, qh = c // 2, c % 2
        out[b, qh * QW:(qh + 1) * QW, :] = res.results[c]["out"]
    return out, res


def kernel(**inputs):
    out, _ = run(trace=False, **inputs)
    return out


# revision 21
# speedup vs baseline: 1.0723x; 1.0657x over previous
"""Multi-head attention Trainium2 kernel (8 NeuronCores, SPMD).

Problem: B=4, S=2048, D=1024, H=16, HD=64 dense MHA with 0/1 mask applied
to scores BEFORE softmax (masked score -> 0, so exp -> 1).

Sharding: core c handles batch b = c//2 and query-row half qh = c%2
(1024 queries). K/V are computed per-batch on both cores of a batch pair
(duplicated) so NO collectives are needed; each core holds every head for
its query rows, so the Wo projection is fully local.

Per-core algorithm (all matmuls bf16, f32 PSUM accumulation):
  KT[e,k] / QT[e,q] projections in transposed layout (head pairs packed
  to 128 partitions). Scores are computed transposed ([k, q]) for a head
  PAIR into one combined PSUM tile [128, h0|h1 x 512]: the two K=64
  matmuls hit disjoint PE row groups (partitions 0-63 / 64-127) and
  disjoint PSUM banks, so they execute concurrently, and a single
  exp + single copy_predicated (masked -> 1.0, exact semantics; the
  inverted-mask tile is column-doubled on the host to cover both heads)
  serve the whole pair. V is kept in natural [k,he] layout with a ones
  column per head (V') so the softmax denominator falls out of the
  attn@V matmul (zT row 64). attn@V runs in zT orientation (lhsT=V'
  chunk, rhs=masked-exp slice, N=512) - no transposes anywhere. All
  biases are applied by K=1 ones-vector matmuls accumulating into the
  projection PSUM. Normalization: denominator row broadcast across 64
  partitions via a K=1 matmul, fast approximate reciprocal (18-bit),
  multiply on GpSimd. Output projection consumes zT directly as lhsT.
"""

import sys

sys.path.insert(0, "/opt/trn_rl_repo")

import numpy as np
import ml_dtypes

import concourse.bass as bass
import concourse.mybir as mybir
import concourse.tile as tile
from concourse import bacc
from concourse.bass_utils import run_bass_kernel_spmd

BF16 = ml_dtypes.bfloat16

B, S, D, H, HD = 4, 2048, 1024, 16, 64
QW = 1024          # queries per core
SK = 2048          # keys per core
NPAIR = 8          # head pairs (2 heads of 64 -> 128 partitions)
DC = 8             # contraction chunks of 128 over D
KC = 16            # key chunks of 128
VW = 65            # V width incl. ones column
QB = 512           # query block (per head) in the paired scores tile
N_CORES = 8

_CACHED_NC = None


def _build_nc():
    dt = mybir.dt
    f32, b16 = dt.float32, dt.bfloat16
    Copy = mybir.ActivationFunctionType.Copy
    Ident = mybir.ActivationFunctionType.Identity
    Exp = mybir.ActivationFunctionType.Exp
    Alu = mybir.AluOpType

    nc = bacc.Bacc("TRN2", target_bir_lowering=False, debug=False)

    d_xqt = nc.dram_tensor("xqt", [D, QW], b16, kind="ExternalInput").ap()
    d_xkt = nc.dram_tensor("xkt", [D, SK], b16, kind="ExternalInput").ap()
    d_xvt = nc.dram_tensor("xvt", [D, SK], b16, kind="ExternalInput").ap()
    # inverted mask, transposed, each 512-wide q block doubled (h0|h1)
    d_im2 = nc.dram_tensor("im2", [SK, 2 * QW], dt.uint8, kind="ExternalInput").ap()
    d_wq = nc.dram_tensor("wq", [D, D], b16, kind="ExternalInput").ap()
    d_wk = nc.dram_tensor("wk", [D, D], b16, kind="ExternalInput").ap()
    d_wv = nc.dram_tensor("wv", [D, D], b16, kind="ExternalInput").ap()
    d_wo = nc.dram_tensor("wo", [D, D], b16, kind="ExternalInput").ap()
    d_bq = nc.dram_tensor("bq", [128, NPAIR], f32, kind="ExternalInput").ap()
    d_bk = nc.dram_tensor("bk", [128, NPAIR], f32, kind="ExternalInput").ap()
    d_bv = nc.dram_tensor("bv", [1, D], b16, kind="ExternalInput").ap()
    d_bob = nc.dram_tensor("bob", [128, D], f32, kind="ExternalInput").ap()
    d_out = nc.dram_tensor("out", [QW, D], f32, kind="ExternalOutput").ap()

    with tile.TileContext(nc) as tc:
        # Keep single-tile free closures alive and idempotent: pools are a
        # LIFO bump allocator per space/side and a GC-run release corrupts
        # the stack order (or lands after scheduling).
        _keep = []

        def single(shape, dtype, name):
            t, free = tc.tile(shape, dtype, name=name)
            done = [False]

            def free_once():
                if not done[0]:
                    done[0] = True
                    free()

            _keep.append(free_once)
            return t, free_once

        # ---------------- persistent SBUF tiles ----------------
        kt, _ = single([128, NPAIR * SK], b16, "kt")       # [pair-e, k] per pair
        qt_, _ = single([128, NPAIR * QW], b16, "qt")      # [pair-e, q] per pair
        vp, _ = single([128, KC * H * VW], b16, "vp")      # [k-chunk, h*65] per chunk
        ones1, _ = single([1, 128], b16, "ones1")          # K=1 lhsT for bias matmuls
        onew, _ = single([128, 2 * QB], b16, "onew")       # ones data for masking
        bqp, _ = single([128, NPAIR], f32, "bqp")
        bkp, _ = single([128, NPAIR], f32, "bkp")
        bvr, _ = single([1, D], b16, "bvr")
        bob_sb, _ = single([128, D], f32, "bob_sb")

        nc.vector.memset(ones1[:], 1.0)
        nc.vector.memset(onew[:], 1.0)
        # ones columns of V' (position 64 of each 65-wide head block)
        nc.vector.memset(vp[:, 64::65], 1.0)

        nc.sync.dma_start(bqp[:], d_bq[:])
        nc.sync.dma_start(bkp[:], d_bk[:])
        nc.sync.dma_start(bvr[:], d_bv[:])
        nc.sync.dma_start(bob_sb[:], d_bob[:])

        # Phase-chained big buffers; frees must be LIFO per space/side.
        xk_sb, xk_free = single([128, DC * SK], b16, "xk_sb")
        wk_sb, wk_free = single([128, DC * D], b16, "wk_sb")
        xv_sb, xv_free = single([128, DC * SK], b16, "xv_sb")
        wv_sb, wv_free = single([128, DC * D], b16, "wv_sb")

        for dc in range(DC):
            nc.sync.dma_start(xv_sb[:, dc * SK:(dc + 1) * SK], d_xvt[dc * 128:(dc + 1) * 128, :])
            nc.sync.dma_start(wv_sb[:, dc * D:(dc + 1) * D], d_wv[dc * 128:(dc + 1) * 128, :])
        for dc in range(DC):
            nc.sync.dma_start(xk_sb[:, dc * SK:(dc + 1) * SK], d_xkt[dc * 128:(dc + 1) * 128, :])
            nc.sync.dma_start(wk_sb[:, dc * D:(dc + 1) * D], d_wk[dc * 128:(dc + 1) * 128, :])

        # ---------------- projections ----------------
        with tc.tile_pool(name="proj_ps", space="PSUM", bufs=4) as proj_pool:
            # V projection: V'[s, h*65:h*65+64] = xv.T chunks @ Wv + bv
            for sc in range(KC):
                for nh in range(2):
                    ps = proj_pool.tile([128, 512], f32, tag="ps")
                    nc.tensor.matmul(  # bias: ones[s] x bv[he]
                        ps[:], lhsT=ones1[:, 0:128],
                        rhs=bvr[:, nh * 512:(nh + 1) * 512],
                        start=True, stop=False,
                    )
                    for dc in range(DC):
                        nc.tensor.matmul(
                            ps[:],
                            lhsT=xv_sb[:, dc * SK + sc * 128: dc * SK + (sc + 1) * 128],
                            rhs=wv_sb[:, dc * D + nh * 512: dc * D + (nh + 1) * 512],
                            start=False, stop=(dc == DC - 1),
                        )
                    # scatter 8 heads x 64 into the 65-strided V' block
                    o3 = vp[:, sc * H * VW + nh * 8 * VW: sc * H * VW + (nh * 8 + 8) * VW]
                    o3 = o3.rearrange("p (h c) -> p h c", h=8)[:, :, 0:64]
                    i3 = ps[:].rearrange("p (h c) -> p h c", h=8)
                    nc.scalar.activation(o3, i3, Copy)
            wv_free()
            xv_free()

            # K projection -> KT [pair-e 128, k]
            for p in range(NPAIR):
                for ns in range(SK // 512):
                    ps = proj_pool.tile([128, 512], f32, tag="ps")
                    for dc in range(DC):
                        nc.tensor.matmul(
                            ps[:],
                            lhsT=wk_sb[:, dc * D + p * 128: dc * D + (p + 1) * 128],
                            rhs=xk_sb[:, dc * SK + ns * 512: dc * SK + (ns + 1) * 512],
                            start=(dc == 0), stop=(dc == DC - 1),
                        )
                    nc.scalar.activation(
                        kt[:, p * SK + ns * 512: p * SK + (ns + 1) * 512],
                        ps[:], Ident, bias=bkp[:, p: p + 1],
                    )
            wk_free()
            xk_free()

            xq_sb, xq_free = single([128, DC * QW], b16, "xq_sb")
            wq_sb, wq_free = single([128, DC * D], b16, "wq_sb")
            for dc in range(DC):
                nc.sync.dma_start(xq_sb[:, dc * QW:(dc + 1) * QW], d_xqt[dc * 128:(dc + 1) * 128, :])
                nc.sync.dma_start(wq_sb[:, dc * D:(dc + 1) * D], d_wq[dc * 128:(dc + 1) * 128, :])

            # Q projection -> QT [pair-e 128, q]
            for p in range(NPAIR):
                for ns in range(QW // 512):
                    ps = proj_pool.tile([128, 512], f32, tag="ps")
                    for dc in range(DC):
                        nc.tensor.matmul(
                            ps[:],
                            lhsT=wq_sb[:, dc * D + p * 128: dc * D + (p + 1) * 128],
                            rhs=xq_sb[:, dc * QW + ns * 512: dc * QW + (ns + 1) * 512],
                            start=(dc == 0), stop=(dc == DC - 1),
                        )
                    nc.scalar.activation(
                        qt_[:, p * QW + ns * 512: p * QW + (ns + 1) * 512],
                        ps[:], Ident, bias=bqp[:, p: p + 1],
                    )
            wq_free()
            xq_free()

        # column-doubled inverted mask into the space freed by x buffers
        im_sb, _ = single([128, KC * 2 * QW], dt.uint8, "im_sb")
        for kc in range(KC):
            nc.sync.dma_start(
                im_sb[:, kc * 2 * QW:(kc + 1) * 2 * QW],
                d_im2[kc * 128:(kc + 1) * 128, :],
            )
        wo_sb, _ = single([128, NPAIR * D], b16, "wo_sb")
        for p in range(NPAIR):
            nc.sync.dma_start(wo_sb[:, p * D:(p + 1) * D], d_wo[p * 128:(p + 1) * 128, :])
        zt, _ = single([128, NPAIR * QW], b16, "zt")       # [pair-he, q]

        # ---------------- attention ----------------
        with (
            tc.tile_pool(name="sc_ps", space="PSUM", bufs=3) as sc_pool,
            tc.tile_pool(name="zt_ps", space="PSUM", bufs=1) as zt_pool,
            tc.tile_pool(name="att_sb", bufs=6) as att_pool,
            tc.tile_pool(name="ep_sb", bufs=3) as ep_pool,
        ):
            for p in range(NPAIR):
                for qb in range(QW // QB):
                    q0 = p * QW + qb * QB
                    zt_ps = [
                        zt_pool.tile([VW, QB], f32, name=f"ztp{hi}", tag=f"ztp{hi}")
                        for hi in range(2)
                    ]
                    for kc in range(KC):
                        # paired scores: both heads' K=64 matmuls in one
                        # combined tile - disjoint row groups AND banks,
                        # gated by the same slot event -> concurrent
                        sc2 = sc_pool.tile([128, 2 * QB], f32, tag="sc")
                        for hi in range(2):
                            r0, r1 = hi * 64, (hi + 1) * 64
                            nc.tensor.matmul(
                                sc2[:, hi * QB:(hi + 1) * QB],
                                lhsT=kt[r0:r1, p * SK + kc * 128: p * SK + (kc + 1) * 128],
                                rhs=qt_[r0:r1, q0: q0 + QB],
                                start=True, stop=True,
                            )
                        e2 = att_pool.tile([128, 2 * QB], b16, tag="e")
                        nc.scalar.activation(e2[:], sc2[:], Exp)
                        # masked positions (inverted mask nonzero) -> 1.0
                        nc.vector.copy_predicated(
                            e2[:],
                            im_sb[:, kc * 2 * QW + qb * 2 * QB: kc * 2 * QW + (qb + 1) * 2 * QB],
                            onew[:],
                        )
                        for hi in range(2):
                            h = 2 * p + hi
                            nc.tensor.matmul(
                                zt_ps[hi][:],
                                lhsT=vp[:, kc * H * VW + h * VW: kc * H * VW + (h + 1) * VW],
                                rhs=e2[:, hi * QB:(hi + 1) * QB],
                                start=(kc == 0), stop=(kc == KC - 1),
                            )
                    # epilogue: move zT out of PSUM fast, normalize from SBUF
                    for hi in range(2):
                        zu = ep_pool.tile([VW, QB], f32, name=f"zu{hi}", tag=f"zu{hi}")
                        nc.scalar.activation(zu[:], zt_ps[hi][:], Copy)
                        den_b = ep_pool.tile([1, QB], b16, tag="denb")
                        nc.scalar.activation(den_b[:], zu[64:65, :], Copy)
                        db_ps = sc_pool.tile([128, 2 * QB], f32, tag="sc")
                        nc.tensor.matmul(
                            db_ps[0:64, 0:QB],
                            lhsT=ones1[:, 0:64], rhs=den_b[:],
                            start=True, stop=True,
                        )
                        rb_sb = ep_pool.tile([64, QB], f32, tag="rbsb")
                        nc.vector.reciprocal_approx_fast(rb_sb[:], db_ps[0:64, 0:QB])
                        nc.gpsimd.tensor_tensor(
                            zt[hi * 64:(hi + 1) * 64, q0: q0 + QB],
                            zu[0:64, :], rb_sb[:], op=Alu.mult,
                        )

        # ---------------- output projection ----------------
        with (
            tc.tile_pool(name="wo_ps", space="PSUM", bufs=2) as wo_pool,
            tc.tile_pool(name="out_sb", bufs=2) as out_pool,
        ):
            for jq in range(8):
                o_sb = out_pool.tile([128, D], f32, tag="o")
                for n in range(2):
                    ps = wo_pool.tile([128, 512], f32, tag="wo")
                    for p in range(NPAIR):
                        nc.tensor.matmul(
                            ps[:],
                            lhsT=zt[:, p * QW + jq * 128: p * QW + (jq + 1) * 128],
                            rhs=wo_sb[:, p * D + n * 512: p * D + (n + 1) * 512],
                            start=(p == 0), stop=(p == NPAIR - 1),
                        )
                    nc.vector.scalar_tensor_tensor(
                        o_sb[:, n * 512:(n + 1) * 512], ps[:], 0.0,
                        bob_sb[:, n * 512:(n + 1) * 512],
                        op0=Alu.bypass, op1=Alu.add,
                    )
                nc.sync.dma_start(d_out[jq * 128:(jq + 1) * 128, :], o_sb[:])

        # Release remaining singles in LIFO order BEFORE TileContext exit,
        # else GC-driven releases append boundary pseudo-instructions to
        # the already-committed program (walrus aborts on them).
        for f in reversed(_keep):
            f()

    nc.compile()
    return nc


def get_nc():
    global _CACHED_NC
    if _CACHED_NC is None:
        _CACHED_NC = _build_nc()
    return _CACHED_NC


def _prep_in_maps(x_v, x_k, x_q, mask, Wq, bq, Wk, bk, Wv, bv, Wo, bo):
    """Host-side shard + layout prep. Cheap numpy transposes/casts only."""
    wq_f = (np.transpose(Wq, (1, 0, 2)).reshape(D, D) / 8.0).astype(BF16)
    wk_f = np.transpose(Wk, (1, 0, 2)).reshape(D, D).astype(BF16)
    wv_f = np.transpose(Wv, (1, 0, 2)).reshape(D, D).astype(BF16)
    wo_f = Wo.astype(BF16)
    bq_f = np.ascontiguousarray((bq.reshape(NPAIR, 128) / 8.0).T).astype(np.float32)
    bk_f = np.ascontiguousarray(bk.reshape(NPAIR, 128).T).astype(np.float32)
    bv_f = bv.reshape(1, D).astype(BF16)
    bob = np.ascontiguousarray(np.broadcast_to(bo.reshape(1, D), (128, D)), dtype=np.float32)

    in_maps = []
    for c in range(N_CORES):
        b, qh = c // 2, c % 2
        q0 = qh * QW
        im = (1 - mask[b, q0:q0 + QW]).T.astype(np.uint8)     # [SK, QW]
        im2 = np.empty((SK, 2 * QW), np.uint8)                # each 512 block doubled
        for qb in range(QW // QB):
            blk = im[:, qb * QB:(qb + 1) * QB]
            im2[:, qb * 2 * QB: qb * 2 * QB + QB] = blk
            im2[:, qb * 2 * QB + QB: (qb + 1) * 2 * QB] = blk
        in_maps.append({
            "xqt": np.ascontiguousarray(x_q[b, q0:q0 + QW].T).astype(BF16),
            "xkt": np.ascontiguousarray(x_k[b].T).astype(BF16),
            "xvt": np.ascontiguousarray(x_v[b].T).astype(BF16),
            "im2": im2,
            "wq": wq_f, "wk": wk_f, "wv": wv_f, "wo": wo_f,
            "bq": bq_f, "bk": bk_f, "bv": bv_f, "bob": bob,
        })
    return in_maps


def _install_axon_ntff_hook():
    """The container's antenv stub lacks axon_hooks, so trace=True can't
    find the NTFF profile hook. Recreate the registry module and install
    the ctypes-based hook from trn_agent_boot against libaxon_pjrt.so."""
    import types

    if "antenv.axon_hooks" in sys.modules:
        return
    import antenv

    mod = types.ModuleType("antenv.axon_hooks")
    _hook = [None]
    mod.set_axon_ntff_profile_hook = lambda h: _hook.__setitem__(0, h)
    mod.get_axon_ntff_profile_hook = lambda: _hook[0]
    sys.modules["antenv.axon_hooks"] = mod
    antenv.axon_hooks = mod
    try:
        sys.path.insert(0, "/root/.axon_site")
        from trn_agent_boot.trn_boot import _ntff_profile_via_ctypes

        mod.set_axon_ntff_profile_hook(
            _ntff_profile_via_ctypes("/opt/axon/libaxon_pjrt.so")
        )
    except Exception as e:  # degrade to no-trace
        print(f"ntff hook install failed: {e}", file=sys.stderr)


def run(trace=False, **inputs):
    if trace:
        _install_axon_ntff_hook()
    nc = get_nc()
    in_maps = _prep_in_maps(**inputs)
    res = run_bass_kernel_spmd(nc, in_maps, core_ids=list(range(N_CORES)), trace=trace)
    out = np.zeros((B, S, D), np.float32)
    for c in range(N_CORES):
        b, qh = c // 2, c % 2
        out[b, qh * QW:(qh + 1) * QW, :] = res.results[c]["out"]
    return out, res


def kernel(**inputs):
    out, _ = run(trace=False, **inputs)
    return out
